# revision 6
# baseline (speedup 1.0000x reference)
"""Channel-attention kernel for Trainium2 (8 NeuronCores, SPMD).

Reference computation (B=2, C=512, H=W=64, heads=8, hd=64, N=H*W=4096):
    tokens = x.transpose(0,2,3,1).reshape(B,N,C)
    qkv    = tokens @ w_qkv.T -> q,k,v per head    (k scaled by hd**-0.5)
    attn   = softmax(k @ v.T, axis=-1)             # [B,h,N,N]
    out    = attn @ q                              # [B,h,N,hd]
    out -> (B,N,h,hd) -> (B,H,W,C) -> (B,C,H,W) -> reshape (B,N,C)
    y      = out @ w_proj.T + b_proj -> reshape (B,C,H,W)

Sharding: 16 (batch, head) pairs over 8 cores -> each core handles one
batch element and two adjacent heads (head-separable end to end, incl.
the projection, thanks to the raw (B,C,H,W)->(B,N,C) reinterpretation).

Per-core pipeline (v2, rebuilt around measured engine rooflines):
  * S^T = (K V^T)^T per (head, m-block, n-chunk) via ONE fp8e4 DoubleRow
    matmul: k-tile slot 0 = (k8, v8), slot 1 = (dk8, v8) where dk8 is the
    fp8 residual of k (k quantization dominates the attention error, so
    the "spare" DoubleRow slot carries its first-order correction).
    256 PE cycles per [128,512] tile instead of 512 (fp32r).
  * exp() is the machine bottleneck (ACT does 1 elem/lane/cycle, no fast
    mode).  Tiles are split between ACT (true exp, fused *hd^-0.5 scale)
    and DVE (Schraudolph bf16: i16 = round(A*S+B) bitcast to bf16, which
    tensor_scalar emits in a single pass).  GPSIMD cannot touch PSUM so
    it only mirrors the vTz slot copy.
  * O^T accumulates over m in PSUM with bf16 operands (E, q); the 65th
    lhsT column of ones accumulates the softmax denominator Z for free.
  * Per-n-chunk O^T is copied to SBUF (ACT), PE-transposed in 128-column
    strips, scaled by 1/Z (reciprocal on DVE, Copy*scale on ACT) into the
    proj-ready M^T layout, then Y = M @ w_proj.T + b_proj streams out.
  * Q is computed channel-major (Q^T) with 512-wide f32r matmuls and
    PE-transposed to token-major: f32r matmuls with <256 output columns
    run 4x slower (moving-operand fetch bound), so the naive token-major
    Q matmul is a trap.
  * PSUM budget (8 banks): 5-bank S ring (manually slotted so exp can
    batch adjacent pairs), 2-bank O accumulator, 1 bank for transposes.
  * x is DMA'd in 64 256-column pieces, quarter-major, so the first
    QKV matmuls start ~6us in instead of waiting for the full 8MB.
"""

import numpy as np

import concourse.bass as bass
import concourse.mybir as mybir
import concourse.tile as tile
from concourse import bacc, bass_utils
from concourse.bass import ts
from concourse.masks import make_identity

F32 = mybir.dt.float32
F32R = mybir.dt.float32r
BF16 = mybir.dt.bfloat16
FP8 = mybir.dt.float8e4
I16 = mybir.dt.int16
EXP = mybir.ActivationFunctionType.Exp
COPY = mybir.ActivationFunctionType.Copy
DR = mybir.MatmulPerfMode.DoubleRow
MULT = mybir.AluOpType.mult
ADD = mybir.AluOpType.add
SUB = mybir.AluOpType.subtract

B, C, H, W = 2, 512, 64, 64
N = H * W                 # 4096
HEADS_TOTAL = 8
HD = C // HEADS_TOTAL     # 64
SCALE = HD ** -0.5        # folded into exp(), NOT into wk (fp8 k stays full-range)
N_CORES = 8
HPC = 2                   # heads per core
NB = N // 128             # 32 m-blocks
NJ = N // 512             # 8 n-chunks
CC = C // 128             # 4 contraction chunks
LAG = 4                   # O-matmul pairs lag behind exp pairs
NSLOT = 5                 # S psum ring slots (banks)
# Schraudolph constants: bf16 bits of 2^(log2e*SCALE*S) ~= exp(S*SCALE)
A8 = 128.0 * 1.4426950408889634 * SCALE
B8 = 128.0 * (127.0 - 0.045)


def r(ap):
    """float32r view for plain-f32 PE operands (bit-identical, faster)."""
    return ap.bitcast(F32R) if ap.dtype == F32 else ap


def _emit(nc, tc):
    x_h = nc.dram_tensor("x", [C, N], F32R, kind="ExternalInput")
    wq_h = nc.dram_tensor("wq", [C, 128], F32R, kind="ExternalInput")
    wk_h = nc.dram_tensor("wk", [C, 128], F32R, kind="ExternalInput")
    wv_h = nc.dram_tensor("wv", [C, 128], F32R, kind="ExternalInput")
    wp_h = nc.dram_tensor("wp", [C, C], F32R, kind="ExternalInput")
    bp_h = nc.dram_tensor("bp", [1, C], F32, kind="ExternalInput")
    out_h = nc.dram_tensor("out", [HPC, 512, 512], F32, kind="ExternalOutput")

    singles = tc.alloc_tile_pool(name="singles", bufs=1)
    psing = tc.alloc_tile_pool(name="psing", bufs=1, space="PSUM")
    epool = tc.alloc_tile_pool(name="epool", bufs=7)
    vpool = tc.alloc_tile_pool(name="vpool", bufs=4)

    # ---- persistent SBUF tensors ----
    x_sb = singles.tile([128, CC, N], F32R)        # x[cc*128+p, n]
    wq_sb = singles.tile([128, CC, 128], F32R)
    wk_sb = singles.tile([128, CC, 128], F32R)
    wv_sb = singles.tile([128, CC, 128], F32R)
    wp_sb = singles.tile([128, CC, 512], F32R)
    bias_sb = singles.tile([128, 512], F32)
    id_sb = singles.tile([128, 128], F32)
    # DoubleRow operand layouts (partitions: head0 rows 0-63, head1 64-127)
    kTz = singles.tile([128, 2, N], FP8)           # slot0 = k8, slot1 = dk8
    vTz = singles.tile([128, 2, N], FP8)           # slot0 = slot1 = v8
    qTs = singles.tile([128, N], F32)              # Q^T channel-major staging
    qa = [singles.tile([128, NB, HD + 1], BF16, name=f"qa{h}") for h in range(HPC)]
    o_all = singles.tile([HD + 1, HPC, N], F32)    # O^T (+Z row) per head
    # f32r: feeds the proj matmul, which requires f32r-rounded producers
    mt = [singles.tile([128, CC, 512], F32R, name=f"mt{h}") for h in range(HPC)]

    # ---- persistent PSUM (manually slotted; 5 + 2 + 1 = 8 banks) ----
    s_big = psing.tile([128, NSLOT, 512], F32)     # S ring / QKV staging
    o_big = psing.tile([128, HPC, 512], F32)       # O^T accumulators
    t_tile = psing.tile([128, 4, HD + 1], F32)     # transpose staging

    make_identity(nc, id_sb)
    for h in range(HPC):
        nc.vector.memset(qa[h][:, :, HD:HD + 1], 1.0)

    # ---- input DMAs: small weights first, then x quarter-major ----
    nc.sync.dma_start(out=wq_sb, in_=wq_h.ap().rearrange("(cc p) m -> p cc m", p=128))
    nc.sync.dma_start(out=wk_sb, in_=wk_h.ap().rearrange("(cc p) m -> p cc m", p=128))
    nc.sync.dma_start(out=wv_sb, in_=wv_h.ap().rearrange("(cc p) m -> p cc m", p=128))
    x_view = x_h.ap().rearrange("(cc p) n -> p cc n", p=128)
    for q in range(4):
        for cc in range(CC):
            for piece in range(4):
                c0 = q * 1024 + piece * 256
                nc.sync.dma_start(
                    out=x_sb[:, cc, c0:c0 + 256], in_=x_view[:, cc, c0:c0 + 256]
                )
    nc.sync.dma_start(out=wp_sb, in_=wp_h.ap().rearrange("(cc p) m -> p cc m", p=128))
    nc.sync.dma_start(out=bias_sb, in_=bp_h.ap().to_broadcast((128, 512)))

    slot_c = [0]  # rolling S-ring cursor

    def next_slot():
        s = slot_c[0] % NSLOT
        slot_c[0] += 1
        return s

    # ---- QKV phase ----
    def kv_group(w_sb, is_k, j8):
        kv_ps = s_big[:, next_slot(), :]
        for cc in range(CC):
            nc.tensor.matmul(
                kv_ps,
                lhsT=r(w_sb[:, cc, :]),
                rhs=r(x_sb[:, cc, ts(j8, 512)]),
                start=(cc == 0),
                stop=(cc == CC - 1),
            )
        dz = kTz if is_k else vTz
        nc.vector.tensor_copy(out=dz[:, 0, ts(j8, 512)], in_=kv_ps)
        if is_k:  # slot1 = fp8 residual of k
            nc.vector.tensor_tensor(
                out=dz[:, 1, ts(j8, 512)], in0=kv_ps, in1=dz[:, 0, ts(j8, 512)], op=SUB
            )
        else:     # slot1 = copy of v8 (SBUF->SBUF, GPSIMD is otherwise idle)
            nc.gpsimd.tensor_copy(out=dz[:, 1, ts(j8, 512)], in_=dz[:, 0, ts(j8, 512)])

    def qT_group(j8):
        qt_ps = s_big[:, next_slot(), :]
        for cc in range(CC):
            nc.tensor.matmul(
                qt_ps,
                lhsT=r(wq_sb[:, cc, :]),
                rhs=r(x_sb[:, cc, ts(j8, 512)]),
                start=(cc == 0),
                stop=(cc == CC - 1),
            )
        nc.vector.tensor_copy(out=qTs[:, ts(j8, 512)], in_=qt_ps)

    def qa_group(nb):
        # token-major q via PE transpose of qTs (f32r matmuls with <256
        # output columns are 4x slower, so q cannot be made token-major
        # directly at full speed)
        tq = o_big[:, (nb // 4) % 2, ts(nb % 4, 128)]
        nc.tensor.transpose(tq, qTs[:, ts(nb, 128)], id_sb)
        for h in range(HPC):
            nc.scalar.activation(
                out=qa[h][:, nb, 0:HD], in_=tq[:, ts(h, HD)], func=COPY
            )

    for q4 in range(4):
        j0 = 2 * q4
        kv_group(wv_sb, False, j0)
        kv_group(wv_sb, False, j0 + 1)
        qT_group(j0)
        for nb in range(8 * q4, 8 * q4 + 4):
            qa_group(nb)
        kv_group(wk_sb, True, j0)
        kv_group(wk_sb, True, j0 + 1)
        qT_group(j0 + 1)
        for nb in range(8 * q4 + 4, 8 * q4 + 8):
            qa_group(nb)

    # ---- attention: S (fp8 DoubleRow) -> exp (ACT|DVE) -> O (bf16) ----
    e_ring = {}
    pending_T = []

    def emit_exp(eng, s0, s1, e_t, adjacent):
        if eng == "A":
            if adjacent:
                nc.scalar.activation(
                    out=e_t, in_=s_big[:, s0:s0 + 2, :], func=EXP, scale=SCALE
                )
            else:
                for t, s in enumerate((s0, s1)):
                    nc.scalar.activation(
                        out=e_t[:, t, :], in_=s_big[:, s, :], func=EXP, scale=SCALE
                    )
        else:
            eo = e_t.bitcast(I16)
            if adjacent:
                nc.vector.tensor_scalar(
                    out=eo, in0=s_big[:, s0:s0 + 2, :],
                    scalar1=A8, scalar2=B8, op0=MULT, op1=ADD,
                )
            else:
                for t, s in enumerate((s0, s1)):
                    nc.vector.tensor_scalar(
                        out=eo[:, t, :], in0=s_big[:, s, :],
                        scalar1=A8, scalar2=B8, op0=MULT, op1=ADD,
                    )

    def emit_o(i):
        e_t = e_ring.pop(i)
        for h in range(HPC):
            nc.tensor.matmul(
                o_big[0:HD + 1, h, :],
                lhsT=r(qa[h][:, i, :]),
                rhs=e_t[:, h, :],
                start=(i == 0),
                stop=(i == NB - 1),
            )

    def emit_transpose(h, q32):
        sl = q32 % 4
        t_ps = t_tile[:, sl, :]
        nc.tensor.transpose(
            t_ps, o_all[:, h, ts(q32, 128)], id_sb[0:HD + 1, 0:HD + 1]
        )
        rz = vpool.tile([128, 1], F32, tag="rz", name="rz")
        nc.vector.reciprocal(out=rz, in_=t_ps[:, HD:HD + 1])
        nc.scalar.activation(
            out=mt[h][:, q32 % 4, (q32 // 4)::8], in_=t_ps[:, 0:HD],
            func=COPY, scale=rz,
        )

    for j in range(NJ):
        myT = pending_T
        pending_T = []
        ti = 0
        for i in range(NB):
            s0 = next_slot()
            s1 = next_slot()
            adjacent = s1 == s0 + 1
            for h, s in ((0, s0), (1, s1)):
                nc.tensor.matmul(
                    s_big[:, s, :],
                    lhsT=vTz[ts(h, HD), :, ts(i, 128)],
                    rhs=kTz[ts(h, HD), :, ts(j, 512)],
                    start=True,
                    stop=True,
                    perf_mode=DR,
                )
            e_t = epool.tile([128, 2, 512], BF16, tag="e", name="e_t")
            emit_exp("A" if i % 2 == 0 else "D", s0, s1, e_t, adjacent)
            e_ring[i] = e_t
            if i >= LAG:
                emit_o(i - LAG)
            if i >= 4 and ti < len(myT):
                emit_transpose(*myT[ti])
                ti += 1
        for i in range(NB - LAG, NB):
            emit_o(i)
        while ti < len(myT):
            emit_transpose(*myT[ti])
            ti += 1
        # O^T (+Z) out of PSUM for both heads in one ACT pass
        nc.scalar.activation(
            out=o_all[:, :, ts(j, 512)], in_=o_big[0:HD + 1, :, :], func=COPY
        )
        pending_T.extend((h, j * 4 + c) for h in range(HPC) for c in range(4))
    for hq in pending_T:
        emit_transpose(*hq)

    # ---- projection ----
    for h in range(HPC):
        for l in range(4):
            y_ps = s_big[:, next_slot(), :]
            for kk in range(CC):
                nc.tensor.matmul(
                    y_ps,
                    lhsT=r(mt[h][:, kk, ts(l, 128)]),
                    rhs=r(wp_sb[:, kk, :]),
                    start=(kk == 0),
                    stop=(kk == CC - 1),
                )
            y_sb = vpool.tile([128, 512], F32, tag="y", name="y_sb")
            nc.vector.tensor_add(out=y_sb, in0=y_ps, in1=bias_sb)
            nc.sync.dma_start(out=out_h.ap()[h, ts(l, 128), :], in_=y_sb)

    for pool in (vpool, epool, psing, singles):
        pool.release()


_CACHE = {}


def _build():
    if "nc" not in _CACHE:
        nc = bacc.Bacc("TRN2", target_bir_lowering=False, debug=False)
        with tile.TileContext(nc) as tc:
            _emit(nc, tc)
        nc.compile()
        _CACHE["nc"] = nc
    return _CACHE["nc"]


def _shard(x, w_qkv, w_proj, b_proj):
    """Build the 8 per-core input maps from the full inputs."""
    wpT = np.ascontiguousarray(w_proj.T)
    bp = np.ascontiguousarray(b_proj.reshape(1, C))
    in_maps = []
    for core in range(N_CORES):
        b = core // 4
        h0 = HPC * (core % 4)
        r0 = h0 * HD
        in_maps.append({
            "x": np.ascontiguousarray(x[b].reshape(C, N)),
            "wq": np.ascontiguousarray(w_qkv[r0:r0 + 128, :].T),
            # NOTE: k left unscaled (hd**-0.5 folded into exp) so fp8
            # quantization sees full-range values
            "wk": np.ascontiguousarray(w_qkv[C + r0:C + r0 + 128, :].T),
            "wv": np.ascontiguousarray(w_qkv[2 * C + r0:2 * C + r0 + 128, :].T),
            "wp": wpT,
            "bp": bp,
        })
    return in_maps


def _gather(results):
    full = np.empty((B, C, N), dtype=np.float32)
    for core in range(N_CORES):
        b = core // 4
        h0 = HPC * (core % 4)
        y = results[core]["out"]  # [2, 512, 512]
        for hi in range(HPC):
            ch0 = (h0 + hi) * HD
            full[b, ch0:ch0 + HD] = y[hi].reshape(HD, N)
    return full.reshape(B, C, H, W)


def run(inputs, trace=False, **kw):
    nc = _build()
    in_maps = _shard(**inputs)
    res = bass_utils.run_bass_kernel_spmd(
        nc, in_maps, core_ids=list(range(N_CORES)), trace=trace, **kw
    )
    return _gather(res.results), res


def kernel(x, w_qkv, w_proj, b_proj):
    out, _ = run(dict(x=x, w_qkv=w_qkv, w_proj=w_proj, b_proj=b_proj))
    return out
